# revision 1
# baseline (speedup 1.0000x reference)
"""
Trainium2 Bass kernel for DirectRankingModel:
    h = tanh(x @ W1.T + b1); s = (h @ W2.T + b2); e = exp(s)
    out = e / segment_sum(e, T)[T]    with 2 segments, N = 2,000,000 rows.

Strategy (8 NeuronCores, data-parallel over rows):
  - Host: block-transpose x into [nblk, 64 feat, 128 rows] so each DMA
    descriptor moves contiguous 512B runs and the PE receives the
    feature-on-partition (transposed) operand directly.  Host also builds
    f32 masks m0/m1 = (T==0)/(T==1) (zero on padded rows).
  - Device per core (R = 262144 rows, padded; 8 super-tiles of 128x256):
      * SWDGE DMA with f32->f16 cast loads "xx" mega tiles [128, 2048]:
        partitions = (half, feature), free = rows.
      * mm1: two K=64 matmuls per 1024 rows (row-split PE: partitions 0-63
        and 64-127 run concurrently), W1T stationary -> PSUM hT [128h, 1024r].
      * tanh on the scalar engine with fused +b1 bias, PSUM -> SBUF fp16.
      * mm2: score s = W2 . h per row, laid out as [128 blocks, 256 rows]:
        32 strip matrices [128, 32] with W2 embedded in column c accumulate
        block b's scores into PSUM partition b (avoids a [1, N] layout).
      * exp with fused +b2 bias -> E [128, 2048] f32 stays SBUF-resident.
      * masked sums via tensor_tensor_reduce, partition_all_reduce, then a
        2-float HBM AllReduce across the 8 cores.
      * normalize: out = E * (inv0 + m1*(inv1-inv0)) and DMA out.
"""

import os
import sys

import numpy as np

for _p in ("/opt/trn_rl_repo", "/root/.axon_site/_ro/trn_rl_repo"):
    if os.path.isdir(_p) and _p not in sys.path:
        sys.path.insert(0, _p)

import concourse.bacc as bacc
import concourse.bass as bass
import concourse.tile as tile
from concourse import bass_isa, mybir
from concourse.bass_utils import run_bass_kernel_spmd

F16 = mybir.dt.float16
F32 = mybir.dt.float32
ALU = mybir.AluOpType
ACTF = mybir.ActivationFunctionType

N_CORES = 8
N_ROWS = 2_000_000
IN_DIM = 64
HID = 128

# Device-side geometry (per core).
Q = 256                 # rows per score-block (mm2 moving free dim)
N_ST = 8                # super-tiles per core; ST = 128 blocks x Q rows = 32768
MEGA_BLK = 32           # x blocks (128 rows each) per mega DMA tile -> 4096 rows
R_CORE = N_ST * 128 * Q          # 262144 rows per core
NBLK_CORE = R_CORE // 128        # 2048
N_PAD = N_CORES * R_CORE         # 2097152 rows total (padded)
NBLK_TOT = N_PAD // 128          # 16384
NBLK_REAL = N_ROWS // 128        # 15625

_MEGAS_PER_ST = (128 * Q) // (MEGA_BLK * 128)   # 8
_SUB_PER_MEGA = (MEGA_BLK * 128) // 1024        # 4  (1024-row mm1 pairs)
_BLOCKS_PER_MEGA = (MEGA_BLK * 128) // Q        # 16 (mm2 blocks per mega)


def _ap(handle_ap, offset, dims):
    """Custom access pattern on a DRAM tensor: dims = [[step, count], ...]."""
    return bass.AP(tensor=handle_ap.tensor, offset=offset, ap=list(dims))


def build_nc(n_st=N_ST, n_cores=N_CORES, use_coll=True, stage=9):
    """Build the per-core Bass program (SPMD: same program, sliced inputs)."""
    from contextlib import ExitStack

    r_core = n_st * 128 * Q
    nblk = r_core // 128
    cols = n_st * Q            # E/mask/out columns per partition

    n_mega = r_core // (MEGA_BLK * 128)

    nc = bacc.Bacc(num_devices=n_cores)

    x_in = nc.declare_dram_parameter(
        "x", [n_mega, 128, MEGA_BLK * 64], F32, isOutput=False
    )
    m0_in = nc.declare_dram_parameter("m0", [r_core], F32, isOutput=False)
    m1_in = nc.declare_dram_parameter("m1", [r_core], F32, isOutput=False)
    w1t_in = nc.declare_dram_parameter("w1t", [IN_DIM, HID], F16, isOutput=False)
    w2s_in = nc.declare_dram_parameter("w2s", [HID, 32 * 32], F16, isOutput=False)
    b1_in = nc.declare_dram_parameter("b1", [HID], F32, isOutput=False)
    b2_in = nc.declare_dram_parameter("b2", [1], F32, isOutput=False)
    out_t = nc.declare_dram_parameter("out", [r_core], F32, isOutput=True)
    gs_t = nc.declare_dram_parameter("gsums", [2], F32, isOutput=True)

    cc_in = nc.dram_tensor("cc_in", [2], F32)
    cc_out = nc.dram_tensor("cc_out", [2], F32, addr_space="Shared")

    B_ELEM = IN_DIM * 128  # elements per x block

    with ExitStack() as ctx:
        tc = ctx.enter_context(tile.TileContext(nc))
        singles = ctx.enter_context(tc.tile_pool(name="singles", bufs=1))
        xx_pool = ctx.enter_context(tc.tile_pool(name="xx", bufs=3))
        ht_pool = ctx.enter_context(tc.tile_pool(name="ht", bufs=3))
        ph_pool = ctx.enter_context(tc.tile_pool(name="ph", bufs=3, space="PSUM"))
        ps_pool = ctx.enter_context(tc.tile_pool(name="ps", bufs=1, space="PSUM"))

        # ---- static setup ----------------------------------------------
        w1t_sb = singles.tile([128, HID], F16)     # both halves hold W1T
        nc.sync.dma_start(
            out=w1t_sb[:],
            in_=_ap(w1t_in[:], 0, [[0, 2], [HID, IN_DIM], [1, HID]]),
        )
        b1_sb = singles.tile([128, 1], F32)
        nc.sync.dma_start(out=b1_sb[:], in_=_ap(b1_in[:], 0, [[1, HID], [1, 1]]))
        b2_sb = singles.tile([128, 1], F32)
        nc.sync.dma_start(out=b2_sb[:], in_=_ap(b2_in[:], 0, [[0, 128], [1, 1]]))

        # 32 strip matrices [128, 32] fp16, strip c has W2 in column c.
        strips = singles.tile([128, 32, 32], F16)
        nc.sync.dma_start(
            out=strips[:], in_=_ap(w2s_in[:], 0, [[32 * 32, HID], [1, 32 * 32]])
        )

        # Masks + persistent E (all f32, SBUF-resident for the whole kernel).
        m0_sb = singles.tile([128, cols], F32)
        m1_sb = singles.tile([128, cols], F32)
        mask_dims = [[Q, 128], [128 * Q, n_st], [1, Q]]
        nc.sync.dma_start(out=m0_sb[:], in_=_ap(m0_in[:], 0, mask_dims))
        nc.sync.dma_start(out=m1_sb[:], in_=_ap(m1_in[:], 0, mask_dims))
        e_sb = singles.tile([128, cols], F32)
        scratch = singles.tile([128, cols], F32)
        out_sb = singles.tile([128, cols], F32)
        rr = singles.tile([128, 2], F32)
        rr_red = singles.tile([128, 2], F32)
        ones_sb = singles.tile([128, 1], F32)
        nc.vector.memset(ones_sb[:], 1.0)
        g_sb = singles.tile([128, 2], F32)
        inv = singles.tile([128, 2], F32)
        dinv = singles.tile([128, 1], F32)

        # ---- phase 1: matmuls / tanh / scores / exp --------------------
        for st in range(n_st):
            s_ps = ps_pool.tile([128, Q], F32, tag="score")
            for m in range(_MEGAS_PER_ST):
                mega = st * _MEGAS_PER_ST + m
                half = MEGA_BLK * 64  # 2048 rows: partition halves g=0/1
                xx = xx_pool.tile([128, half], F16, tag="xx")
                src = _ap(
                    x_in[:],
                    mega * 128 * half,
                    [[half, 128], [1, half]],
                )
                nc.gpsimd.dma_start(out=xx[:], in_=src)  # f32 -> f16 cast DMA

                ht = ht_pool.tile([128, MEGA_BLK * 128], F16, tag="ht")
                for t in range(_SUB_PER_MEGA):
                    ph = ph_pool.tile([128, 1024], F32, tag="ph")
                    nc.tensor.matmul(
                        ph[:, 0:512],
                        w1t_sb[0:64, :],
                        xx[0:64, t * 512 : (t + 1) * 512],
                        start=True,
                        stop=True,
                    )
                    nc.tensor.matmul(
                        ph[:, 512:1024],
                        w1t_sb[64:128, :],
                        xx[64:128, t * 512 : (t + 1) * 512],
                        start=True,
                        stop=True,
                    )
                    # ht col layout is (t, g, j): col = t*1024 + g*512 + j,
                    # holding row mega_base + g*2048 + t*512 + j.
                    nc.scalar.activation(
                        out=ht[:, t * 1024 : (t + 1) * 1024],
                        in_=ph[:, 0:1024],
                        func=ACTF.Tanh,
                        bias=b1_sb[:],
                        scale=1.0,
                    )
                for bl in range(_BLOCKS_PER_MEGA):
                    b = m * _BLOCKS_PER_MEGA + bl
                    c = b % 32
                    g = b // 32
                    # rows bl*256..+256 of this mega live at ht col offset:
                    hoff = ((bl % 8) // 2) * 1024 + (bl // 8) * 512 + (bl % 2) * Q
                    nc.tensor.matmul(
                        s_ps[32 * g : 32 * g + 32, :],
                        strips[:, c, :],
                        ht[:, hoff : hoff + Q],
                        start=(c == 0),
                        stop=(c == 31),
                        skip_group_check=True,
                        tile_position=(0, 32 * g),
                    )
            nc.scalar.activation(
                out=e_sb[:, st * Q : (st + 1) * Q],
                in_=s_ps[:],
                func=ACTF.Exp,
                bias=b2_sb[:],
                scale=1.0,
            )

        # ---- segment sums + allreduce ----------------------------------
        if stage <= 1:
            # phase-1 only: dump E and a dummy gsums
            nc.sync.dma_start(
                out=_ap(out_t[:], 0, [[Q, 128], [128 * Q, n_st], [1, Q]]),
                in_=e_sb[:],
            )
            nc.sync.dma_start(out=gs_t[:], in_=e_sb[0:1, 0:2])
            nc.compile()
            return nc
        nc.vector.tensor_mul(scratch[:], e_sb[:], m0_sb[:])
        nc.vector.reduce_sum(rr[:, 0:1], scratch[:], axis=mybir.AxisListType.X)
        nc.vector.tensor_mul(scratch[:], e_sb[:], m1_sb[:])
        nc.vector.reduce_sum(rr[:, 1:2], scratch[:], axis=mybir.AxisListType.X)
        if stage <= 2:
            # skip partition reduce: use per-partition sums (wrong values)
            nc.vector.tensor_copy(rr_red[:], rr[:])
        else:
            # cross-partition sum via ones-matmul (PE), [128,2] -> [1,2]
            ps_rr = ps_pool.tile([128, 2], F32, tag="score")
            nc.tensor.matmul(
                ps_rr[0:1, :], ones_sb[:], rr[:], start=True, stop=True
            )
            nc.scalar.activation(
                out=rr_red[0:1, :],
                in_=ps_rr[0:1, :],
                func=ACTF.Copy,
                bias=0.0,
                scale=1.0,
            )
        if use_coll:
            nc.gpsimd.dma_start(out=cc_in[:], in_=rr_red[0:1, :])
            nc.gpsimd.collective_compute(
                "AllReduce",
                ALU.add,
                replica_groups=[list(range(n_cores))],
                ins=[cc_in[:]],
                outs=[cc_out[:]],
            )
            nc.sync.dma_start(out=gs_t[:], in_=cc_out[:])
            nc.sync.dma_start(
                out=g_sb[:], in_=_ap(cc_out[:], 0, [[0, 128], [1, 2]])
            )
        else:
            nc.sync.dma_start(out=gs_t[:], in_=rr_red[0:1, :])
            nc.vector.tensor_copy(g_sb[:], rr_red[:])

        # ---- normalize + store -----------------------------------------
        nc.vector.reciprocal(out=inv[:], in_=g_sb[:])
        nc.vector.tensor_sub(dinv[:], inv[:, 1:2], inv[:, 0:1])
        nc.vector.tensor_scalar(
            out=scratch[:],
            in0=m1_sb[:],
            scalar1=dinv[:],
            scalar2=inv[:, 0:1],
            op0=ALU.mult,
            op1=ALU.add,
        )
        nc.vector.tensor_mul(out_sb[:], scratch[:], e_sb[:])
        nc.sync.dma_start(
            out=_ap(out_t[:], 0, [[Q, 128], [128 * Q, n_st], [1, Q]]),
            in_=out_sb[:],
        )

    nc.compile()
    return nc


_NC_CACHE = {}


def _get_nc(n_st=N_ST):
    if n_st not in _NC_CACHE:
        _NC_CACHE[n_st] = build_nc(n_st=n_st)
    return _NC_CACHE[n_st]


def prep_inputs(x, T, W1, b1, W2, b2, n_st=N_ST, n_cores=N_CORES):
    """Host-side shard/layout prep -> per-core input maps."""
    r_core = n_st * 128 * Q
    nblk = r_core // 128
    n_pad = n_cores * r_core
    n_rows = x.shape[0]
    nblk_real = n_rows // 128

    x = np.ascontiguousarray(np.asarray(x, dtype=np.float32))
    rows_mega = MEGA_BLK * 128                      # 4096
    half = rows_mega // 2                           # 2048
    n_mega_tot = n_pad // rows_mega
    n_full = n_rows // rows_mega
    xd = np.zeros((n_mega_tot, 128, half), dtype=np.float32)
    xd[:n_full] = (
        x[: n_full * rows_mega]
        .reshape(n_full, 2, half, IN_DIM)
        .transpose(0, 1, 3, 2)
        .reshape(n_full, 128, half)
    )
    rem = n_rows - n_full * rows_mega
    if rem:
        r0 = min(rem, half)
        xd[n_full, :IN_DIM, :r0] = x[n_full * rows_mega :][:r0].T
        if rem > half:
            xd[n_full, IN_DIM:, : rem - half] = x[n_full * rows_mega + half :].T
    n_mega_core = n_mega_tot // n_cores

    T = np.asarray(T)
    m0 = np.zeros(n_pad, dtype=np.float32)
    m1 = np.zeros(n_pad, dtype=np.float32)
    m0[:n_rows] = T == 0
    m1[:n_rows] = T == 1

    w1t = np.ascontiguousarray(np.asarray(W1, np.float32).T).astype(np.float16)
    w2s = np.zeros((HID, 32, 32), dtype=np.float16)
    w2v = np.asarray(W2, np.float32).reshape(HID).astype(np.float16)
    for c in range(32):
        w2s[:, c, c] = w2v
    w2s = w2s.reshape(HID, 32 * 32)
    b1h = np.asarray(b1, np.float32).reshape(HID).copy()
    b2h = np.asarray(b2, np.float32).reshape(1).copy()

    in_maps = []
    for cid in range(n_cores):
        in_maps.append(
            {
                "x": xd[cid * n_mega_core : (cid + 1) * n_mega_core],
                "m0": m0[cid * r_core : (cid + 1) * r_core],
                "m1": m1[cid * r_core : (cid + 1) * r_core],
                "w1t": w1t,
                "w2s": w2s,
                "b1": b1h,
                "b2": b2h,
            }
        )
    return in_maps


def run(x, T, W1, b1, W2, b2, n_st=N_ST, trace=False):
    in_maps = prep_inputs(x, T, W1, b1, W2, b2, n_st=n_st)
    nc = _get_nc(n_st)
    res = run_bass_kernel_spmd(nc, in_maps, list(range(N_CORES)), trace=trace)
    out = np.concatenate([res.results[c]["out"] for c in range(N_CORES)])
    return out[: x.shape[0]].astype(np.float32, copy=False), res


def kernel(x, T, W1, b1, W2, b2):
    out, _ = run(x, T, W1, b1, W2, b2)
    return out



# revision 4
# speedup vs baseline: 1.1207x; 1.1207x over previous
"""
Trainium2 Bass kernel for DirectRankingModel:
    h = tanh(x @ W1.T + b1); s = (h @ W2.T + b2); e = exp(s)
    out = e / segment_sum(e, T)[T]    with 2 segments, N = 2,000,000 rows.

Strategy (8 NeuronCores, data-parallel over rows; v2):
  - Host: cast x to f16 and block-transpose into chunks of 8192 rows:
    xd[ch] = [128, 4096] where partitions 0-63 hold features of rows
    [0,4096) and 64-127 hold rows [4096,8192) -> HWDGE DMA (1 MiB each),
    row-split PE runs both halves concurrently.
  - 62 "megas" (253952 rows) per core: only 1.6% padding (vs 4.9%).
  - mm1: K=64 f16 matmuls, N=512, pairs on partition halves -> PSUM
    ph [128, 1536] (3 banks, 2 bufs).
  - tanh on ACT in 1536-wide instructions (PSUM->SBUF f16) -- ACT is the
    kernel's roofline (~0.95 ns/col/core).
  - mm2: 32-strip trick; block b -> psum partition b%128, strips cycle
    c=b%32 with col-group tile_position; exp per 128-block super-tile.
  - Segment sums: single sel mask (1.0 where T==1) + per-core pad
    correction scalars; per-ST incremental DVE sums; ones-matmul
    partition reduce; 2-float HBM AllReduce across cores.
  - normalize: out = E * (inv0 + sel*(inv1-inv0)) and one 1 MiB store.
"""

import os
import sys

import numpy as np

for _p in ("/opt/trn_rl_repo", "/root/.axon_site/_ro/trn_rl_repo"):
    if os.path.isdir(_p) and _p not in sys.path:
        sys.path.insert(0, _p)

import concourse.bacc as bacc
import concourse.bass as bass
import concourse.tile as tile
from concourse import mybir
from concourse.bass_utils import run_bass_kernel_spmd

F16 = mybir.dt.float16
F32 = mybir.dt.float32
ALU = mybir.AluOpType
ACTF = mybir.ActivationFunctionType

N_CORES = 8
N_ROWS = 2_000_000
IN_DIM = 64
HID = 128

# Device-side geometry (per core).
Q = 256                     # rows per score-block
CH_ROWS = 8192              # rows per DMA chunk ([128, 4096] f16 = 1 MiB)
N_CH = 31                   # chunks per core
R_CORE = N_CH * CH_ROWS     # 253952 rows per core
N_PAD = N_CORES * R_CORE    # 2031616 rows total (padded)
MM_PER_CH = 16              # mm1 matmuls (512 rows each) per chunk
N_MM = N_CH * MM_PER_CH     # 496
PH_MMS = 3                  # mm1 outputs per PSUM tile -> ACT N=1536
PH_COLS = 512 * PH_MMS
BLK_PER_HT = PH_COLS // Q   # 6 score blocks per ht tile
N_BLK = R_CORE // Q         # 992
N_ST = (N_BLK + 127) // 128  # 8 super-tiles (last partial: 96 blocks)
COLS = N_ST * Q             # 2048 e/sel/out columns per partition


def _ap(handle_ap, offset, dims):
    """Custom access pattern on a DRAM tensor: dims = [[step, count], ...]."""
    return bass.AP(tensor=handle_ap.tensor, offset=offset, ap=list(dims))


def build_nc(n_cores=N_CORES, use_coll=True, stage=9):
    """Build the per-core Bass program (SPMD: same program, sliced inputs)."""
    from contextlib import ExitStack

    nc = bacc.Bacc(num_devices=n_cores)

    x_in = nc.declare_dram_parameter("x", [N_CH, 128, 4096], F16, isOutput=False)
    sel_in = nc.declare_dram_parameter("sel", [128 * COLS], F32, isOutput=False)
    w1t_in = nc.declare_dram_parameter("w1t", [IN_DIM, HID], F16, isOutput=False)
    w2s_in = nc.declare_dram_parameter("w2s", [HID, 32 * 32], F16, isOutput=False)
    b1_in = nc.declare_dram_parameter("b1", [HID], F32, isOutput=False)
    b2_in = nc.declare_dram_parameter("b2", [1], F32, isOutput=False)
    padc_in = nc.declare_dram_parameter("padc", [2], F32, isOutput=False)
    out_t = nc.declare_dram_parameter("out", [128 * COLS], F32, isOutput=True)
    gs_t = nc.declare_dram_parameter("gsums", [2], F32, isOutput=True)

    cc_in = nc.dram_tensor("cc_in", [2], F32)
    cc_out = nc.dram_tensor("cc_out", [2], F32, addr_space="Shared")

    with ExitStack() as ctx:
        tc = ctx.enter_context(tile.TileContext(nc))
        singles = ctx.enter_context(tc.tile_pool(name="singles", bufs=1))
        xx_pool = ctx.enter_context(tc.tile_pool(name="xx", bufs=3))
        ht_pool = ctx.enter_context(tc.tile_pool(name="ht", bufs=4))
        ph_pool = ctx.enter_context(tc.tile_pool(name="ph", bufs=2, space="PSUM"))
        ps_pool = ctx.enter_context(tc.tile_pool(name="ps", bufs=1, space="PSUM"))

        # ---- static setup ----------------------------------------------
        # First x chunk before anything else on the HWDGE queue.
        xx_tiles = {}
        xx_tiles[0] = xx_pool.tile([128, 4096], F16, tag="xx", name="xx")
        nc.sync.dma_start(
            out=xx_tiles[0][:], in_=_ap(x_in[:], 0, [[4096, 128], [1, 4096]])
        )

        w1t_sb = singles.tile([128, HID], F16)     # both halves hold W1T
        nc.sync.dma_start(
            out=w1t_sb[:],
            in_=_ap(w1t_in[:], 0, [[0, 2], [HID, IN_DIM], [1, HID]]),
        )
        b1_sb = singles.tile([128, 1], F32)
        nc.sync.dma_start(out=b1_sb[:], in_=_ap(b1_in[:], 0, [[1, HID], [1, 1]]))
        b2_sb = singles.tile([128, 1], F32)
        nc.sync.dma_start(out=b2_sb[:], in_=_ap(b2_in[:], 0, [[0, 128], [1, 1]]))

        # 32 strip matrices [128, 32] fp16, strip c has W2 in column c.
        strips = singles.tile([128, 32, 32], F16)
        nc.sync.dma_start(
            out=strips[:], in_=_ap(w2s_in[:], 0, [[32 * 32, HID], [1, 32 * 32]])
        )
        pc_sb = singles.tile([1, 2], F32)
        nc.sync.dma_start(out=pc_sb[:], in_=_ap(padc_in[:], 0, [[2, 1], [1, 2]]))

        sel_sb = singles.tile([128, COLS], F32)
        nc.sync.dma_start(
            out=sel_sb[:], in_=_ap(sel_in[:], 0, [[COLS, 128], [1, COLS]])
        )

        e_sb = singles.tile([128, COLS], F32)
        out_sb = singles.tile([128, COLS], F32)
        scr = singles.tile([128, Q], F32)
        rr_sel = singles.tile([128, N_ST], F32)
        rr_tot = singles.tile([128, N_ST], F32)
        rr2 = singles.tile([128, 2], F32)
        rr_red = singles.tile([128, 2], F32)
        ones_sb = singles.tile([128, 1], F32)
        tiny = singles.tile([128, 1], F32)
        g2 = singles.tile([1, 2], F32)
        g_sb = singles.tile([128, 2], F32)
        inv = singles.tile([128, 2], F32)
        dinv = singles.tile([128, 1], F32)

        nc.vector.memset(ones_sb[:], 1.0)
        # Zero the unused corner of E (last super-tile has 96 blocks).
        nc.vector.memset(e_sb[96:128, (N_ST - 1) * Q : N_ST * Q], 0.0)
        # Dummy activation: pulls ACT_TABLE_LOAD off the critical path.
        nc.scalar.activation(
            out=tiny[:], in_=ones_sb[:], func=ACTF.Tanh, bias=0.0, scale=1.0
        )

        # ---- phase 1: mm1 / tanh / mm2 / exp ---------------------------
        # Global mm1 index m: chunk ch=m//16, j=m%16 -> half=j%2, t=j//2.
        # R-order rows [512m, 512(m+1)) = chunk rows half*4096 + 512t ...
        ph = None
        ht = None
        s_ps = None
        next_blk = 0

        def emit_mm2_upto(bmax):
            nonlocal s_ps, next_blk
            while next_blk < bmax:
                b = next_blk
                c = b % 32
                g = (b % 128) // 32
                if b % 128 == 0:
                    s_ps = ps_pool.tile([128, Q], F32, tag="score")
                k = b // BLK_PER_HT
                hoff = Q * (b % BLK_PER_HT)
                nc.tensor.matmul(
                    s_ps[32 * g : 32 * g + 32, :],
                    strips[:, c, :],
                    ht_tiles[k][:, hoff : hoff + Q],
                    start=(c == 0),
                    stop=(c == 31),
                    skip_group_check=True,
                    tile_position=(0, 32 * g),
                )
                next_blk += 1
                if next_blk % 128 == 0 or next_blk == N_BLK:
                    st = (next_blk - 1) // 128
                    npart = 128 if next_blk % 128 == 0 else (next_blk - st * 128)
                    nc.scalar.activation(
                        out=e_sb[0:npart, st * Q : (st + 1) * Q],
                        in_=s_ps[0:npart, :],
                        func=ACTF.Exp,
                        bias=b2_sb[0:npart, :],
                        scale=1.0,
                    )
                    # incremental segment sums for this super-tile (DVE)
                    nc.vector.tensor_mul(
                        scr[:], e_sb[:, st * Q : (st + 1) * Q],
                        sel_sb[:, st * Q : (st + 1) * Q],
                    )
                    nc.vector.reduce_sum(
                        rr_sel[:, st : st + 1], scr[:], axis=mybir.AxisListType.X
                    )
                    nc.vector.reduce_sum(
                        rr_tot[:, st : st + 1],
                        e_sb[:, st * Q : (st + 1) * Q],
                        axis=mybir.AxisListType.X,
                    )

        ht_tiles = {}
        for m in range(N_MM):
            ch, j = divmod(m, MM_PER_CH)
            if j == 0 and ch not in xx_tiles:
                xx_tiles[ch] = xx_pool.tile([128, 4096], F16, tag="xx", name="xx")
                nc.sync.dma_start(
                    out=xx_tiles[ch][:],
                    in_=_ap(
                        x_in[:], ch * 128 * 4096, [[4096, 128], [1, 4096]]
                    ),
                )
            half, t = j % 2, j // 2
            slot = m % PH_MMS
            if slot == 0:
                ph = ph_pool.tile([128, PH_COLS], F32, tag="ph")
            nc.tensor.matmul(
                ph[:, 512 * slot : 512 * (slot + 1)],
                w1t_sb[64 * half : 64 * half + 64, :],
                xx_tiles[ch][64 * half : 64 * half + 64, 512 * t : 512 * (t + 1)],
                start=True,
                stop=True,
            )
            if slot == PH_MMS - 1 or m == N_MM - 1:
                k = m // PH_MMS
                ncols = 512 * (slot + 1)
                ht_tiles[k] = ht_pool.tile([128, PH_COLS], F16, tag="ht", name="ht")
                nc.scalar.activation(
                    out=ht_tiles[k][:, 0:ncols],
                    in_=ph[:, 0:ncols],
                    func=ACTF.Tanh,
                    bias=b1_sb[:],
                    scale=1.0,
                )
                emit_mm2_upto(min((k * PH_COLS + ncols) // Q, N_BLK))
                # free old chunks implicitly via pool reuse

        # ---- segment sums + allreduce ----------------------------------
        if stage <= 1:
            nc.sync.dma_start(
                out=_ap(out_t[:], 0, [[COLS, 128], [1, COLS]]), in_=e_sb[:]
            )
            nc.sync.dma_start(out=gs_t[:], in_=e_sb[0:1, 0:2])
            nc.compile()
            return nc

        # rr_sel/rr_tot [128, 8] -> rr2 [128, 2] -> partition reduce -> [1,2]
        nc.vector.reduce_sum(rr2[:, 0:1], rr_tot[:], axis=mybir.AxisListType.X)
        nc.vector.reduce_sum(rr2[:, 1:2], rr_sel[:], axis=mybir.AxisListType.X)
        ps_rr = ps_pool.tile([128, 2], F32, tag="score")
        nc.tensor.matmul(ps_rr[0:1, :], ones_sb[:], rr2[:], start=True, stop=True)
        nc.scalar.activation(
            out=rr_red[0:1, :], in_=ps_rr[0:1, :], func=ACTF.Copy, bias=0.0,
            scale=1.0,
        )
        # g2 = [sum0, sum1]: sum1 = sel_sum - padc[1];
        #                    sum0 = (tot - sel_sum) - padc[0]
        nc.vector.tensor_sub(g2[0:1, 1:2], rr_red[0:1, 1:2], pc_sb[0:1, 1:2])
        nc.vector.tensor_sub(g2[0:1, 0:1], rr_red[0:1, 0:1], rr_red[0:1, 1:2])
        nc.vector.tensor_sub(g2[0:1, 0:1], g2[0:1, 0:1], pc_sb[0:1, 0:1])

        if use_coll:
            nc.gpsimd.dma_start(out=cc_in[:], in_=g2[0:1, :])
            nc.gpsimd.collective_compute(
                "AllReduce",
                ALU.add,
                replica_groups=[list(range(n_cores))],
                ins=[cc_in[:]],
                outs=[cc_out[:]],
            )
            nc.sync.dma_start(out=gs_t[:], in_=cc_out[:])
            nc.sync.dma_start(
                out=g_sb[:], in_=_ap(cc_out[:], 0, [[0, 128], [1, 2]])
            )
        else:
            nc.sync.dma_start(out=gs_t[:], in_=g2[0:1, :])
            nc.sync.dma_start(
                out=g_sb[:], in_=_ap(gs_t[:], 0, [[0, 128], [1, 2]])
            )

        # ---- normalize + store -----------------------------------------
        nc.vector.reciprocal(out=inv[:], in_=g_sb[:])
        nc.vector.tensor_sub(dinv[:], inv[:, 1:2], inv[:, 0:1])
        nc.vector.tensor_scalar(
            out=out_sb[:],
            in0=sel_sb[:],
            scalar1=dinv[:],
            scalar2=inv[:, 0:1],
            op0=ALU.mult,
            op1=ALU.add,
        )
        nc.vector.tensor_mul(out_sb[:], out_sb[:], e_sb[:])
        nc.sync.dma_start(
            out=_ap(out_t[:], 0, [[COLS, 128], [1, COLS]]), in_=out_sb[:]
        )

    nc.compile()
    return nc


_NC_CACHE = {}


def _get_nc(key=0):
    if key not in _NC_CACHE:
        _NC_CACHE[key] = build_nc()
    return _NC_CACHE[key]


# Host-side index map (identical for every core): for actual row a in
# [0, R_CORE): position in the device (p, col) layout.
_IDX_CACHE = {}


def _layout_index():
    """Return (p, col) arrays mapping core-row a -> device layout slot."""
    if "idx" not in _IDX_CACHE:
        a = np.arange(R_CORE, dtype=np.int64)
        ch, o = a // CH_ROWS, a % CH_ROWS
        half, w = o // 4096, o % 4096
        t, jr = w // 512, w % 512
        m = MM_PER_CH * ch + 2 * t + half
        q = 512 * m + jr
        b, r = q // Q, q % Q
        p = b % 128
        col = (b // 128) * Q + r
        _IDX_CACHE["idx"] = (p, col)
    return _IDX_CACHE["idx"]


def prep_inputs(x, T, W1, b1, W2, b2, n_cores=N_CORES):
    """Host-side shard/layout prep -> per-core input maps."""
    n_rows = x.shape[0]
    assert n_rows == N_ROWS

    x = np.asarray(x, dtype=np.float32)
    xh = np.zeros((N_PAD, IN_DIM), dtype=np.float16)
    xh[:n_rows] = x
    # [n_chunk_tot, 2, 4096, 64] -> [n_chunk_tot, 2, 64, 4096] -> [., 128, 4096]
    n_ch_tot = N_PAD // CH_ROWS
    xd = np.ascontiguousarray(
        xh.reshape(n_ch_tot, 2, 4096, IN_DIM)
        .transpose(0, 1, 3, 2)
        .reshape(n_ch_tot, 128, 4096)
    )

    T = np.asarray(T)
    p_idx, c_idx = _layout_index()

    w1t = np.ascontiguousarray(np.asarray(W1, np.float32).T).astype(np.float16)
    w2s = np.zeros((HID, 32, 32), dtype=np.float16)
    w2v = np.asarray(W2, np.float32).reshape(HID).astype(np.float16)
    for c in range(32):
        w2s[:, c, c] = w2v
    w2s = w2s.reshape(HID, 32 * 32)
    b1h = np.asarray(b1, np.float32).reshape(HID).copy()
    b2h = np.asarray(b2, np.float32).reshape(1).copy()

    # Host estimate of the device's E value on padded rows (x = 0).
    h_pad = np.tanh(b1h)
    e_pad = float(np.exp(b2h[0] + np.asarray(W2, np.float32).reshape(HID) @ h_pad))

    in_maps = []
    for cid in range(n_cores):
        base = cid * R_CORE
        n_real = min(max(n_rows - base, 0), R_CORE)
        selv = np.zeros(R_CORE, dtype=np.float32)
        selv[:n_real] = T[base : base + n_real] == 1
        sel_layout = np.zeros((128, COLS), dtype=np.float32)
        sel_layout[p_idx, c_idx] = selv
        n_pad_c = R_CORE - n_real
        padc = np.array([n_pad_c * e_pad, 0.0], dtype=np.float32)
        in_maps.append(
            {
                "x": xd[cid * N_CH : (cid + 1) * N_CH],
                "sel": sel_layout.reshape(-1),
                "w1t": w1t,
                "w2s": w2s,
                "b1": b1h,
                "b2": b2h,
                "padc": padc,
            }
        )
    return in_maps


def run(x, T, W1, b1, W2, b2, trace=False):
    in_maps = prep_inputs(x, T, W1, b1, W2, b2)
    nc = _get_nc()
    res = run_bass_kernel_spmd(nc, in_maps, list(range(N_CORES)), trace=trace)
    p_idx, c_idx = _layout_index()
    n_rows = x.shape[0]
    out = np.empty(n_rows, dtype=np.float32)
    for cid in range(N_CORES):
        base = cid * R_CORE
        n_real = min(max(n_rows - base, 0), R_CORE)
        if n_real <= 0:
            break
        lay = res.results[cid]["out"].reshape(128, COLS)
        out[base : base + n_real] = lay[p_idx[:n_real], c_idx[:n_real]]
    return out, res


def kernel(x, T, W1, b1, W2, b2):
    out, _ = run(x, T, W1, b1, W2, b2)
    return out
